# revision 35
# baseline (speedup 1.0000x reference)
"""BlockDCTSandwich Trainium2 kernel.

The whole op (blockify -> 8x8 DCT -> zigzag gather -> Linear(64,64) -> IDCT
-> deblockify) is a single fused 64x64 linear map per 8x8 block:
    out_vec = M @ x_vec + c,  M = kron(D^T,D^T) @ W @ G @ kron(D,D),
    c = kron(D^T,D^T) @ bias
(everything is linear; G is the gather matrix for the zigzag reorder).

Data parallel: one batch element per NeuronCore; 64 [128,512] tiles per
core. Per tile (row r = 8*hb+n, hb = 4*hbh+hbl; col c = w5*256+wbl*8+m):

  T2a (DVE 32x32 stream transpose, SBUF->SBUF):
      Y[hbh*32+wbl, hbl*128 + w5*64 + m*8 + n] = X[r, c]
  T1b (PE transpose x4, contiguous 128-col chunks j=hbl):
      psZ[w5*64+m*8+n, hbl*128 + hbh*32+wbl] = Y[hbh*32+wbl, hbl*128 + .]
  ACT evac psZ -> Zs (flat copy), then
  MM  (fp32r, moving 512 -> 1 cycle/row): psB = LT2.T @ Zs,
      LT2 block-diag over w5, slot k = w5*64 + m*8 + n.
  T2a' (DVE, PSUM->SBUF):
      Y2[(w5,m'2)*32+wbl, m'lo*128 + hbh*32 + hbl*8 + n'] = psB[jz, .]
  T1b' (PE transpose x4, j2=m'lo):
      psC[8*hb+n', m'lo*128 + w5*64 + m'2*32 + wbl] = Y2[., m'lo*128 + .]
  ACT evac' -> OXB natural row layout, DMA rows out.

Self-contained: hardcodes shapes x=(8,16,512,512) f32, W=(64,64), bias=(64,).
"""

import sys

import numpy as np

if "/opt/trn_rl_repo" not in sys.path:
    sys.path.insert(0, "/opt/trn_rl_repo")

_B = 8
_NCORES = 8

# use single 4-free-dim views where possible (False: split into 3-free-dim
# instruction pairs, which is known-safe for DVE/ACT)
FOUR_FREE = True


def _dct_matrix(b):
    n = np.arange(b)
    k = n[:, None]
    Dm = np.sqrt(2.0 / b) * np.cos(np.pi * (2 * n[None, :] + 1) * k / (2 * b))
    Dm[0] *= 1.0 / np.sqrt(2.0)
    return Dm


def _build_idx(b):
    def to_key(x):
        s = x[0] + x[1]
        o = b * b * s
        if s % 2 == 1:
            o += x[0]
        else:
            o -= x[0]
        return o

    coords = sorted(([i, j] for i in range(b) for j in range(b)), key=to_key)
    arr = np.array(coords).reshape(b, b, 2)
    return (np.arange(b)[None, :] * arr[..., 0] + arr[..., 1]).reshape(-1)


def _consts(W, bias):
    """Fused 64x64 map M as 128x128 block-diagonal stationary lhsT.

    Partition slot on both sides of the MM: k(n, m, w5) = w5*64 + m*8 + n.
    """
    D = _dct_matrix(_B)
    idx = _build_idx(_B)
    G = np.zeros((64, 64))
    G[np.arange(64), idx] = 1.0
    M = np.kron(D.T, D.T) @ W.astype(np.float64) @ G @ np.kron(D, D)
    c = np.kron(D.T, D.T) @ bias.astype(np.float64)

    enc = np.zeros(64, np.int64)  # flat (n,m) -> slot m*8+n
    for n in range(8):
        for m in range(8):
            enc[n * 8 + m] = m * 8 + n
    LT = np.zeros((128, 128), np.float64)
    for w5 in range(2):
        kk = enc + 64 * w5
        LT[np.ix_(kk, kk)] = M.T  # LT[k_in, j_out] = M[out, in]
    return LT.astype(np.float32), c


_NC_CACHE = {}


def _build_nc(ntb=16):
    key = ("nc", ntb, FOUR_FREE)
    if key in _NC_CACHE:
        return _NC_CACHE[key]
    import concourse.bass as bass
    import concourse.mybir as mybir
    from concourse import bacc
    from concourse.tile import TileContext

    f32 = mybir.dt.float32
    f32r = mybir.dt.float32r
    ds = bass.ds

    nc = bacc.Bacc("TRN2", target_bir_lowering=False, debug=False,
                   num_devices=_NCORES)
    xin = nc.dram_tensor("xin", [512 * ntb, 512], f32, kind="ExternalInput")
    ltw = nc.dram_tensor("ltw", [128, 128], f32r, kind="ExternalInput")
    yout = nc.dram_tensor("yout", [512 * ntb, 512], f32, kind="ExternalOutput")

    xin_ap = xin.ap()
    yout_ap = yout.ap()

    with TileContext(nc) as tc:
        with (
            tc.tile_pool(name="wp", bufs=1) as wp,
            tc.tile_pool(name="xt", bufs=4) as xtp,
            tc.tile_pool(name="ot", bufs=4) as otp,
            tc.tile_pool(name="wk", bufs=4) as wk,
            tc.tile_pool(name="pz", bufs=3, space="PSUM") as pzp,
            tc.tile_pool(name="pb", bufs=2, space="PSUM") as pbp,
            tc.tile_pool(name="pc", bufs=3, space="PSUM") as pcp,
        ):
            lt_sb = wp.tile([128, 128], f32r)
            nc.gpsimd.dma_start(out=lt_sb[:, :], in_=ltw.ap())
            id_sb = wp.tile([128, 128], f32, tag="id_sb")
            from concourse import masks as _masks
            _masks.make_identity(nc, id_sb[:, :])

            ntile = 4 * ntb
            XTh, XBh = {}, {}
            Yh, psZh, Zsh, psBh, Y2h, psCh = {}, {}, {}, {}, {}, {}

            def st_load(i):
                # first TB: per-tile loads (fast pipeline rampup); rest: per-TB
                if i < 4:
                    XTh[i] = xtp.tile([128, 512], f32, tag="XT",
                                      name=f"XT{i}")
                    nc.sync.dma_start(out=XTh[i][:, :],
                                      in_=xin_ap[ds(i * 128, 128), :])
                elif i % 4 == 0:
                    TB = i // 4
                    XBh[TB] = xtp.tile([128, 2048], f32, tag="XB",
                                       name=f"XB{TB}")
                    nc.sync.dma_start(
                        out=XBh[TB][:, :],
                        in_=xin_ap[ds(TB * 512, 512), :]
                        .rearrange("(t4 r) w -> t4 r w", t4=4, r=128)
                        .transpose([1, 0, 2]),
                    )

            def st_t2a(i):
                # T2a: Y[(hbh,wbl), hbl*128 + w5*64 + m*8 + n] = X[r, c]
                Yh[i] = wk.tile([128, 512], f32, tag="Y", name=f"Y{i}")
                Yo = Yh[i][:, :].rearrange(
                    "p (hbl w5 m n) -> p w5 m hbl n", hbl=4, w5=2, m=8, n=8)
                if i < 4:
                    Xt = XTh[i][:, :]
                else:
                    Xt = XBh[i // 4][:, ds((i % 4) * 512, 512)]
                Xi = Xt.rearrange("p (w5 wbl m) -> p w5 m wbl",
                                  w5=2, wbl=32, m=8)
                nc.vector.transpose(out=Yo, in_=Xi)
                if i < 4:
                    del XTh[i]
                elif i % 4 == 3:
                    del XBh[i // 4]

            def st_t1b(i):
                psZh[i] = pzp.tile([128, 512], f32, tag="psZ",
                                   name=f"psZ{i}")
                for j in range(4):
                    nc.tensor.matmul(
                        psZh[i][:, ds(128 * j, 128)],
                        Yh[i][:, ds(128 * j, 128)],
                        id_sb[:, :], is_transpose=True,
                    )
                del Yh[i]

            def st_evac(i):
                Zsh[i] = wk.tile([128, 512], f32r, tag="Zs", name=f"Zs{i}")
                nc.scalar.copy(Zsh[i][:, :], psZh[i][:, :])
                del psZh[i]

            def st_mm(i):
                psBh[i] = pbp.tile([128, 512], f32, tag="psB",
                                   name=f"psB{i}")
                nc.tensor.matmul(psBh[i][:, :], lt_sb[:, :], Zsh[i][:, :],
                                 start=True, stop=True)
                del Zsh[i]

            def st_t2ap(i):
                # T2a': Y2[q2, mlo*128 + hbh*32 + hbl*8 + n'] = T(psB)
                Y2h[i] = wk.tile([128, 512], f32, tag="Y2", name=f"Y2{i}")
                Y2o = Y2h[i][:, :].rearrange(
                    "p (mlo hbh hbl n) -> p hbl hbh mlo n",
                    mlo=4, hbh=4, hbl=4, n=8)
                psBi = psBh[i][:, :].rearrange(
                    "p (hbl hbh wbl) -> p hbl hbh wbl", hbl=4, hbh=4, wbl=32)
                nc.vector.transpose(out=Y2o, in_=psBi)
                del psBh[i]

            def st_t1bp(i):
                psCh[i] = pcp.tile([128, 512], f32, tag="psC",
                                   name=f"psC{i}")
                for j2 in range(4):
                    nc.tensor.matmul(
                        psCh[i][:, ds(128 * j2, 128)],
                        Y2h[i][:, ds(128 * j2, 128)],
                        id_sb[:, :], is_transpose=True,
                    )
                del Y2h[i]

            OXBh = {}
            Vh = {}

            def st_evf(i):
                # ACT flat evac': psC (PSUM) -> V (SBUF)
                Vh[i] = wk.tile([128, 512], f32, tag="V", name=f"V{i}")
                nc.scalar.copy(Vh[i][:, :], psCh[i][:, :])
                del psCh[i]

            def st_asm(i):
                # Pool strided assembly (SBUF->SBUF, 2 instrs over w5):
                # OX[p, w5*256+wbl*8+m2*4+mlo] = V[p, mlo*128+w5*64+m2*32+wbl]
                TB, t4 = i // 4, i % 4
                last_tb = i >= ntile - 4
                if last_tb:
                    OT = otp.tile([128, 512], f32, tag="OT", name=f"OT{i}")
                else:
                    if t4 == 0:
                        OXBh[TB] = otp.tile([128, 2048], f32, tag="OXB",
                                            name=f"OXB{TB}")
                    OT = OXBh[TB][:, ds(t4 * 512, 512)]
                Vr = Vh[i][:, :].rearrange(
                    "p (mlo w5 m2 wbl) -> p w5 mlo m2 wbl",
                    mlo=4, w5=2, m2=2, wbl=32)
                for w5 in range(2):
                    OXt = OT[:, ds(w5 * 256, 256)].rearrange(
                        "p (wbl m2 mlo) -> p mlo m2 wbl", wbl=32, m2=2, mlo=4)
                    nc.gpsimd.tensor_copy(OXt, Vr[:, w5])
                del Vh[i]
                if last_tb:
                    # last TB: per-tile stores so the tail drains ASAP
                    nc.scalar.dma_start(out=yout_ap[ds(i * 128, 128), :],
                                        in_=OT[:, :])
                elif t4 == 3:
                    nc.gpsimd.dma_start(
                        out=yout_ap[ds(TB * 512, 512), :]
                        .rearrange("(t4 r) w -> t4 r w", t4=4, r=128)
                        .transpose([1, 0, 2]),
                        in_=OXBh[TB][:, :],
                    )
                    del OXBh[TB]

            for i in range(ntile + 7):
                if i < ntile:
                    st_load(i)
                    st_t2a(i)
                if 0 <= i - 1 < ntile:
                    st_t1b(i - 1)
                if 0 <= i - 2 < ntile:
                    st_evac(i - 2)
                    st_mm(i - 2)
                if 0 <= i - 3 < ntile:
                    st_t2ap(i - 3)
                if 0 <= i - 4 < ntile:
                    st_t1bp(i - 4)
                if 0 <= i - 5 < ntile:
                    st_evf(i - 5)
                if 0 <= i - 6 < ntile:
                    st_asm(i - 6)

    nc.finalize()
    _NC_CACHE[key] = nc
    return nc


def run(x, W, bias, trace=False, ntb=16):
    from concourse.bass_utils import run_bass_kernel_spmd

    x = np.ascontiguousarray(np.asarray(x, dtype=np.float32))
    W = np.asarray(W, dtype=np.float32)
    bias = np.asarray(bias, dtype=np.float32)
    assert x.shape == (8, 16, 512, 512), x.shape

    LT, c = _consts(W, bias)
    nc = _build_nc(ntb)
    in_maps = [
        {"xin": np.ascontiguousarray(x[i, :ntb].reshape(512 * ntb, 512)),
         "ltw": LT}
        for i in range(_NCORES)
    ]
    res = run_bass_kernel_spmd(nc, in_maps, core_ids=list(range(_NCORES)),
                               trace=trace)
    out = np.stack(
        [res.results[i]["yout"].reshape(ntb, 512, 512) for i in range(_NCORES)]
    )
    if np.any(c):
        cimg = np.tile(c.reshape(8, 8), (64, 64)).astype(np.float32)
        out = out + cimg[None, None]
    return out.astype(np.float32), res


def kernel(x, W, bias):
    out, _ = run(x, W, bias, trace=False)
    return out
